# revision 3
# baseline (speedup 1.0000x reference)
"""Attention-LSTM decoder (nn_Decoder) Trainium2 Bass kernel, v2.

Sharding: data-parallel over batch B=64 -> 8 cores x 8 examples (BL=8).
All operands SBUF-resident; T=200 recurrence fully unrolled; no per-step DMA.

Per-step structure (per core):
  gates : col-tiled PE MMs — gate n in PE col-group n, psum rows 32n..32n+7.
          k=0 is a one-hot matmul (K=34) against EW = emb@W_x.T + b (bias and
          embedding folded in); k=1..8 contract ctxT/hT feature chunks.
  lstm  : 4 sigmoid/tanh ACTs, 4 DVE STTs, tanh ACT.
  energy: row-form e[1, s-half] per example, col-tiled 4-way over examples
          (ex m -> psum tile m//4, col-group m%4, row 32*(m%4)); exp via
          strided-partition ACT reads compacting to wm[8, 800].
  softmax: masked-mult + row-sum in one DVE STT (accum_out), reciprocal.
  ctx   : wT via 7 PE transposes; row-form ctx accumulated over 7 s-chunks,
          col-tiled 4-way; normalized via scalar.mul with AP scale.
  h/ctx transposed into resident hT_all/ctxT_all (feature-on-partition).
MLP head is hoisted out of the loop: one batched GEMM pass over all T*8
columns at the end (full 128-partition utilization), logits DMA'd per chunk.
"""

import math
import sys
from contextlib import ExitStack

import numpy as np

sys.path.insert(0, "/opt/trn_rl_repo")

import ml_dtypes  # noqa: E402

import concourse.bass as bass  # noqa: E402
import concourse.bacc as bacc  # noqa: E402
import concourse.tile as tile  # noqa: E402
from concourse import mybir  # noqa: E402
from concourse.masks import make_identity  # noqa: E402

BF16 = ml_dtypes.bfloat16

V, E, H, C = 34, 256, 512, 512
B, T, S = 64, 200, 800
NCORES = 8
BL = B // NCORES
SCALE = 1.0 / math.sqrt(128.0)
FDT = mybir.dt.float32
BDT = mybir.dt.bfloat16

S_HALVES = [(0, 512), (512, 288)]  # (offset, width); each fits one psum bank
NSJ = 7  # s chunks of 128 (last has 32 rows)


def build_bass(t_steps: int = T) -> bass.Bass:
    nc = bacc.Bacc()

    kts_d = nc.dram_tensor("kts", [BL, 4, 128, S], BDT, kind="ExternalInput")
    v_d = nc.dram_tensor("v", [BL, 6, 128, C], BDT, kind="ExternalInput")
    v6_d = nc.dram_tensor("v6", [128, 2, C], BDT, kind="ExternalInput")
    w_d = nc.dram_tensor("wt", [8, 128, 4 * H], BDT, kind="ExternalInput")
    ew_d = nc.dram_tensor("ewt", [V, 4, 512], BDT, kind="ExternalInput")
    oh_d = nc.dram_tensor("oh", [V, t_steps, BL], BDT, kind="ExternalInput")
    w1_d = nc.dram_tensor("w1t", [8, 128, C], BDT, kind="ExternalInput")
    w2_d = nc.dram_tensor("w2t", [4, 128, V], BDT, kind="ExternalInput")
    b1c_d = nc.dram_tensor("b1c", [128, 4], FDT, kind="ExternalInput")
    b2c_d = nc.dram_tensor("b2c", [V, 1], FDT, kind="ExternalInput")
    mask_d = nc.dram_tensor("mask2", [BL, 2, 512], BDT, kind="ExternalInput")
    out_d = nc.dram_tensor("out", [V, t_steps * BL], BDT, kind="ExternalOutput")

    AF = mybir.ActivationFunctionType
    OP = mybir.AluOpType

    with tile.TileContext(nc) as tc, ExitStack() as es:
        consts = es.enter_context(tc.tile_pool(name="consts", bufs=1))
        state = es.enter_context(tc.tile_pool(name="state", bufs=1))
        work = es.enter_context(tc.tile_pool(name="work", bufs=1))
        work2 = es.enter_context(tc.tile_pool(name="work2", bufs=2))
        peA = es.enter_context(tc.tile_pool(name="peA", bufs=1, space="PSUM"))
        peB = es.enter_context(tc.tile_pool(name="peB", bufs=1, space="PSUM"))
        pt = es.enter_context(tc.tile_pool(name="pt", bufs=3, space="PSUM"))

        # ---- resident inputs ----
        kts_sb = consts.tile([128, BL, 4, S], BDT)
        for ex in range(BL):
            for hk in range(4):
                nc.sync.dma_start(out=kts_sb[:, ex, hk, :], in_=kts_d[ex, hk])
        v_sb = consts.tile([128, BL, 6, C], BDT)
        for ex in range(BL):
            for sj in range(6):
                nc.sync.dma_start(out=v_sb[:, ex, sj, :], in_=v_d[ex, sj])
        v6_sb = consts.tile([128, 2, C], BDT)  # [32*(m%4)+r, m//4, c] = V[m, 768+r, c]
        nc.sync.dma_start(out=v6_sb, in_=v6_d[:, :, :])
        w_sb = consts.tile([128, 8, 4 * H], BDT)
        for k in range(8):
            nc.sync.dma_start(out=w_sb[:, k, :], in_=w_d[k])
        ew_sb = consts.tile([V, 4, 512], BDT)
        nc.sync.dma_start(out=ew_sb, in_=ew_d[:, :, :])
        oh_sb = consts.tile([V, t_steps, BL], BDT)
        nc.sync.dma_start(out=oh_sb, in_=oh_d[:, :, :])
        w1_sb = consts.tile([128, 8, C], BDT)
        for k in range(8):
            nc.sync.dma_start(out=w1_sb[:, k, :], in_=w1_d[k])
        w2_sb = consts.tile([128, 4, V], BDT)
        for k in range(4):
            nc.sync.dma_start(out=w2_sb[:, k, :], in_=w2_d[k])
        b1c_sb = consts.tile([128, 4], FDT)
        nc.sync.dma_start(out=b1c_sb, in_=b1c_d[:, :])
        b2c_sb = consts.tile([V, 1], FDT)
        nc.sync.dma_start(out=b2c_sb, in_=b2c_d[:, :])
        mask_sb = consts.tile([BL, 2, 512], BDT)
        nc.sync.dma_start(out=mask_sb, in_=mask_d[:, :, :])
        id8f = consts.tile([BL, BL], FDT)
        make_identity(nc, id8f)
        id8b = consts.tile([BL, BL], BDT)
        nc.vector.tensor_copy(id8b, id8f)

        # ---- recurrent state: feature-on-partition history ----
        hT_all = state.tile([128, 4, (t_steps + 1) * BL], BDT)
        ctxT_all = state.tile([128, 4, (t_steps + 1) * BL], BDT)
        nc.vector.memset(hT_all[:, :, 0:BL], 0.0)
        nc.vector.memset(ctxT_all[:, :, 0:BL], 0.0)

        P_A = peA.tile([128, 2, 512], FDT, tag="e")
        P_B = peB.tile([128, 2, 512], FDT, tag="e")
        nc.vector.memset(P_A, 0.0)
        nc.vector.memset(P_B, 0.0)

        c_prev = None

        for t in range(t_steps):
            sl_in = slice(t * BL, (t + 1) * BL)
            sl_out = slice((t + 1) * BL, (t + 2) * BL)

            # ---- gates: psum [128, 512]; gate n at rows 32n..32n+7 ----
            # (shares the energy psum banks; strictly ordered by the LSTM chain)
            g_ps = P_A[:, 0, :]
            for k in range(9):
                for n in range(4):
                    if k == 0:
                        lhsT = oh_sb[:, t, :]
                        rhs = ew_sb[:, n, :]
                    elif k <= 4:
                        lhsT = ctxT_all[:, k - 1, sl_in]
                        rhs = w_sb[:, k - 1, n * 512 : (n + 1) * 512]
                    else:
                        lhsT = hT_all[:, k - 5, sl_in]
                        rhs = w_sb[:, k - 1, n * 512 : (n + 1) * 512]
                    nc.tensor.matmul(
                        g_ps[32 * n : 32 * n + BL, :],
                        lhsT,
                        rhs,
                        start=(k == 0),
                        stop=(k == 8),
                        tile_position=(0, 32 * n),
                        skip_group_check=True,
                    )

            # ---- LSTM pointwise ----
            gact = [None] * 4
            for n in (2, 0, 1, 3):  # g first: shortens the c-chain
                ga = work.tile([BL, 512], FDT if n == 1 else BDT, tag=f"ga{n}")
                nc.scalar.activation(
                    ga,
                    g_ps[32 * n : 32 * n + BL, :],
                    AF.Tanh if n == 2 else AF.Sigmoid,
                )
                gact[n] = ga

            def tt(out, a, b, op, accum_out=None):
                nc.vector.scalar_tensor_tensor(
                    out=out, in0=a, scalar=1.0, in1=b, op0=OP.mult, op1=op,
                    accum_out=accum_out,
                )

            t_ig = work.tile([BL, H], BDT, tag="t_ig")
            tt(t_ig, gact[0], gact[2], OP.mult)
            c_new = work2.tile([BL, H], FDT, tag="c")
            if c_prev is None:
                nc.vector.tensor_copy(c_new, t_ig)
            else:
                t_fc = work.tile([BL, H], FDT, tag="t_fc")
                tt(t_fc, gact[1], c_prev, OP.mult)
                tt(c_new, t_ig, t_fc, OP.add)
            c_prev = c_new
            tnc = work.tile([BL, H], BDT, tag="tnc")
            nc.scalar.activation(tnc, c_new, AF.Tanh)
            h_sb = work.tile([BL, H], BDT, tag="h_sb")
            tt(h_sb, gact[3], tnc, OP.mult)

            # ---- h^T into hT_all[slot t+1] ----
            for ck in range(4):
                tp = pt.tile([128, BL], BDT, tag="t")
                nc.tensor.transpose(tp, h_sb[:, ck * 128 : (ck + 1) * 128], id8b)
                nc.vector.tensor_copy(hT_all[:, ck, sl_out], tp)

            # ---- attention energy: ex m -> tile m//4, col-group m%4 ----
            eA = P_A
            eB = P_B
            for hi, (soff, sw) in enumerate(S_HALVES):
                for hk in range(4):
                    for m in range(BL):
                        g = m % 4
                        e_ps = eA if m < 4 else eB
                        nc.tensor.matmul(
                            e_ps[32 * g : 32 * g + 1, hi, 0:sw],
                            hT_all[:, hk, (t + 1) * BL + m : (t + 1) * BL + m + 1],
                            kts_sb[:, m, hk, soff : soff + sw],
                            start=(hk == 0),
                            stop=(hk == 3),
                            tile_position=(0, 32 * g),
                            skip_group_check=True,
                        )

            # exp over the full 97-row psum block (garbage rows are finite),
            # then DMA partition-gathers compact rows {0,32,64,96} -> 0..3.
            wmA = work.tile([97, 2, 512], BDT, tag="wmA")
            wmB = work.tile([97, 2, 512], BDT, tag="wmB")
            nc.scalar.activation(wmA, eA[0:97, :, :], AF.Exp, scale=SCALE)
            nc.scalar.activation(wmB, eB[0:97, :, :], AF.Exp, scale=SCALE)
            wmc = work.tile([BL, 2, 512], BDT, tag="wmc")
            nc.sync.dma_start(out=wmc[0:4, :, :], in_=wmA[0:97:32, :, :])
            nc.sync.dma_start(out=wmc[4:8, :, :], in_=wmB[0:97:32, :, :])

            # masked weights + row-sum (mask2 zeroes the pad columns too)
            wmask = work.tile([BL, 2, 512], BDT, tag="wmask")
            wsum = work.tile([BL, 1], FDT, tag="wsum")
            tt(wmask, wmc, mask_sb, OP.mult, accum_out=wsum)
            rinv = work.tile([BL, 1], FDT, tag="rinv")
            nc.vector.reciprocal(rinv, wsum)

            # ---- w^T via PE transpose (sj6 lands at rows 32*(m%4)) ----
            wT = work.tile([128, NSJ, BL], BDT, tag="wT")
            for sj in range(6):
                hi, off = (0, sj * 128) if sj < 4 else (1, (sj - 4) * 128)
                tp = pt.tile([128, BL], BDT, tag="t")
                nc.tensor.transpose(tp, wmask[:, hi, off : off + 128], id8b)
                nc.vector.tensor_copy(wT[:, sj, :], tp)
            tp6 = pt.tile([128, BL], BDT, tag="t")
            nc.tensor.transpose(tp6[0:32, :], wmask[:, 1, 256:288], id8b)
            for g in range(4):
                nc.vector.tensor_copy(
                    wT[32 * g : 32 * g + 32, 6, g::4], tp6[0:32, g::4]
                )

            # ---- ctx: row-form, col-tiled; reuses energy psum banks ----
            cA = P_A
            cB = P_B
            for sj in range(NSJ):
                for m in range(BL):
                    g = m % 4
                    c_ps = cA if m < 4 else cB
                    if sj < 6:
                        lhsT = wT[:, sj, m : m + 1]
                        rhs = v_sb[:, m, sj, :]
                        tpos = (0, 32 * g)
                    else:
                        lhsT = wT[32 * g : 32 * g + 32, 6, m : m + 1]
                        rhs = v6_sb[32 * g : 32 * g + 32, m // 4, :]
                        tpos = (32 * g, 32 * g)
                    nc.tensor.matmul(
                        c_ps[32 * g : 32 * g + 1, 0, :],
                        lhsT,
                        rhs,
                        start=(sj == 0),
                        stop=(sj == NSJ - 1),
                        tile_position=tpos,
                        skip_group_check=True,
                    )
            ctxsA = work.tile([97, C], BDT, tag="ctxsA")
            ctxsB = work.tile([97, C], BDT, tag="ctxsB")
            nc.scalar.copy(ctxsA, cA[0:97, 0, :])
            nc.scalar.copy(ctxsB, cB[0:97, 0, :])
            ctx_u = work.tile([BL, C], BDT, tag="ctx_u")
            nc.sync.dma_start(out=ctx_u[0:4, :], in_=ctxsA[0:97:32, :])
            nc.sync.dma_start(out=ctx_u[4:8, :], in_=ctxsB[0:97:32, :])
            ctx_row = work.tile([BL, C], BDT, tag="ctx_row")
            nc.vector.tensor_scalar_mul(ctx_row, ctx_u, rinv)

            # ---- ctx^T into ctxT_all[slot t+1] ----
            for ck in range(4):
                tp = pt.tile([128, BL], BDT, tag="t")
                nc.tensor.transpose(tp, ctx_row[:, ck * 128 : (ck + 1) * 128], id8b)
                nc.vector.tensor_copy(ctxT_all[:, ck, sl_out], tp)

        # ---- batched MLP head over all t_steps*BL columns ----
        NT = t_steps * BL
        tcs = []
        t0 = 0
        while t0 < NT:
            tcs.append((t0, min(512, NT - t0)))
            t0 += 512
        for ti, (t0, tw) in enumerate(tcs):
            hidT = work.tile([128, 4, 512], BDT, tag="hidT")
            for cj in range(4):
                m_ps = P_A if (ti + cj) % 2 == 0 else P_B
                for k in range(8):
                    src = hT_all if k < 4 else ctxT_all
                    nc.tensor.matmul(
                        m_ps[:, 0, 0:tw],
                        w1_sb[:, k, cj * 128 : (cj + 1) * 128],
                        src[:, k % 4, BL + t0 : BL + t0 + tw],
                        start=(k == 0),
                        stop=(k == 7),
                    )
                nc.scalar.activation(
                    hidT[:, cj, 0:tw],
                    m_ps[:, 0, 0:tw],
                    AF.Tanh,
                    bias=b1c_sb[:, cj : cj + 1],
                )
            l_ps = (P_B if ti % 2 == 0 else P_A)[:, 1, :]
            for ck in range(4):
                nc.tensor.matmul(
                    l_ps[0:V, 0:tw],
                    w2_sb[:, ck, :],
                    hidT[:, ck, 0:tw],
                    start=(ck == 0),
                    stop=(ck == 3),
                )
            o_chunk = work2.tile([V, 512], BDT, tag="o_chunk")
            nc.scalar.activation(
                o_chunk[:, 0:tw], l_ps[0:V, 0:tw], AF.Identity, bias=b2c_sb
            )
            nc.sync.dma_start(out=out_d[:, t0 : t0 + tw], in_=o_chunk[:, 0:tw])

    return nc


def prep_core_inputs(core, tokens, key_enc, value_enc, out_lens, t_steps=T):
    sl = slice(core * BL, (core + 1) * BL)
    ke = key_enc[sl]  # [BL, S, H] f32
    kts = ke.transpose(0, 2, 1).reshape(BL, 4, 128, S).astype(BF16)

    vc = value_enc[sl]  # [BL, S, C]
    v = vc[:, :768].reshape(BL, 6, 128, C).astype(BF16)
    v6 = np.zeros((128, 2, C), np.float32)
    for m in range(BL):
        v6[32 * (m % 4) : 32 * (m % 4) + 32, m // 4] = vc[m, 768:800]

    oh = np.zeros((V, t_steps, BL), np.float32)
    tok = tokens[sl, :t_steps]  # [BL, t]
    for ex in range(BL):
        oh[tok[ex], np.arange(t_steps), ex] = 1.0

    mask = (np.arange(S)[None, :] < out_lens[sl][:, None]).astype(np.float32)
    mask2 = np.zeros((BL, 2, 512), np.float32)
    mask2[:, 0, :] = mask[:, :512]
    mask2[:, 1, :288] = mask[:, 512:]

    return {
        "kts": np.ascontiguousarray(kts),
        "v": np.ascontiguousarray(v),
        "v6": np.ascontiguousarray(v6.astype(BF16)),
        "oh": np.ascontiguousarray(oh.astype(BF16)),
        "mask2": np.ascontiguousarray(mask2.astype(BF16)),
    }


def prep_shared_inputs(emb, W_ih, W_hh, b_ih, b_hh, W1, b1, W2, b2):
    # EW' = emb @ W_x.T + b_ih + b_hh  -> [V, 2048] -> [V, 4, 512]
    ew = emb @ W_ih[:, :E].T + b_ih + b_hh
    wc = np.concatenate([W_ih[:, E:], W_hh], axis=1)  # [2048, 1024]
    wt = wc.T.reshape(8, 128, 4 * H).astype(BF16)
    return {
        "ewt": np.ascontiguousarray(ew.reshape(V, 4, 512).astype(BF16)),
        "wt": np.ascontiguousarray(wt),
        "w1t": np.ascontiguousarray(W1.T.reshape(8, 128, C).astype(BF16)),
        "w2t": np.ascontiguousarray(W2.T.reshape(4, 128, V).astype(BF16)),
        "b1c": np.ascontiguousarray(b1.reshape(4, 128).T.astype(np.float32)),
        "b2c": np.ascontiguousarray(b2[:, None].astype(np.float32)),
    }


_CACHE = {}


def _get_nc(t_steps):
    if t_steps not in _CACHE:
        nc = build_bass(t_steps)
        nc.finalize()
        _CACHE[t_steps] = nc
    return _CACHE[t_steps]


def _build_in_maps(t_steps, inputs):
    args = {k: np.asarray(v) for k, v in inputs.items()}
    tokens = args["tokens"].astype(np.int64)
    shared = prep_shared_inputs(
        args["emb"], args["W_ih"], args["W_hh"], args["b_ih"], args["b_hh"],
        args["W1"], args["b1"], args["W2"], args["b2"],
    )
    in_maps = []
    for core in range(NCORES):
        m = prep_core_inputs(
            core, tokens, args["key_enc"], args["value_enc"], args["out_lens"],
            t_steps=t_steps,
        )
        m.update(shared)
        in_maps.append(m)
    return in_maps


def _unpack_out(outs, t_steps):
    # outs: per-core [V, t_steps*BL] -> [B, t_steps, V]
    full = []
    for o in outs:
        full.append(o.reshape(V, t_steps, BL).transpose(2, 1, 0))
    return np.concatenate(full, axis=0)


def run(t_steps=T, trace=False, **inputs):
    from concourse.bass_utils import run_bass_kernel_spmd

    in_maps = _build_in_maps(t_steps, inputs)
    nc = _get_nc(t_steps)
    res = run_bass_kernel_spmd(nc, in_maps, list(range(NCORES)), trace=trace)
    outs = [np.asarray(r["out"], np.float32) for r in res.results]
    return _unpack_out(outs, t_steps), res


def kernel(**inputs) -> np.ndarray:
    full, _ = run(t_steps=T, trace=False, **inputs)
    return full


def warm_timing(t_steps=T, n_iters=3, **inputs):
    """Time warm NEFF executions (device-resident inputs) as an HW-time proxy."""
    import time

    import jax
    from jax.sharding import Mesh, PartitionSpec
    from jax.experimental.shard_map import shard_map

    from concourse import bass2jax
    from concourse import mybir as _mybir
    from concourse.bass2jax import _bass_exec_p, install_neuronx_cc_hook

    install_neuronx_cc_hook()
    in_maps = _build_in_maps(t_steps, inputs)
    nc = _get_nc(t_steps)

    partition_name = nc.partition_id_tensor.name if nc.partition_id_tensor else None
    in_names, out_names, out_avals, zero_outs = [], [], [], []
    for alloc in nc.m.functions[0].allocations:
        if not isinstance(alloc, _mybir.MemoryLocationSet):
            continue
        name = alloc.memorylocations[0].name
        if alloc.kind == "ExternalInput":
            if name != partition_name:
                in_names.append(name)
        elif alloc.kind == "ExternalOutput":
            out_names.append(name)
            shape = tuple(alloc.tensor_shape)
            dtype = _mybir.dt.np(alloc.dtype)
            out_avals.append(jax.core.ShapedArray(shape, dtype))
            zero_outs.append(np.zeros(shape, dtype))
    n_params = len(in_names)
    n_outs = len(out_avals)
    in_names.extend(out_names)
    if partition_name:
        in_names.append(partition_name)

    def _body(*a):
        operands = list(a)
        if partition_name:
            operands.append(bass2jax.partition_id_tensor())
        return tuple(
            _bass_exec_p.bind(
                *operands,
                out_avals=tuple(out_avals),
                in_names=tuple(in_names),
                out_names=tuple(out_names),
                lowering_input_output_aliases=(),
                sim_require_finite=True,
                sim_require_nnan=True,
                nc=nc,
            )
        )

    devices = jax.devices()[:NCORES]
    mesh = Mesh(np.asarray(devices), ("core",))
    sharded = jax.jit(
        shard_map(
            _body,
            mesh=mesh,
            in_specs=(PartitionSpec("core"),) * (n_params + n_outs),
            out_specs=(PartitionSpec("core"),) * len(out_names),
            check_rep=False,
        ),
        keep_unused=True,
    )
    per_core = [[np.asarray(m[nm]) for nm in in_names[:n_params]] for m in in_maps]
    concat_in = [
        jax.device_put(np.concatenate([per_core[c][i] for c in range(NCORES)], axis=0))
        for i in range(n_params)
    ]
    concat_zeros = [
        jax.device_put(np.zeros((NCORES * z.shape[0], *z.shape[1:]), z.dtype))
        for z in zero_outs
    ]
    outs = sharded(*concat_in, *concat_zeros)
    jax.block_until_ready(outs)
    best = None
    for _ in range(n_iters):
        t0 = time.time()
        outs = sharded(*concat_in, *concat_zeros)
        jax.block_until_ready(outs)
        dt = time.time() - t0
        best = dt if best is None else min(best, dt)

    oarr = np.asarray(outs[out_names.index("out")]).reshape(NCORES, V, t_steps * BL)
    full = _unpack_out(list(oarr), t_steps)
    return best, full


# revision 4
# speedup vs baseline: 1.0505x; 1.0505x over previous
"""Attention-LSTM decoder (nn_Decoder) Trainium2 Bass kernel, v2.

Sharding: data-parallel over batch B=64 -> 8 cores x 8 examples (BL=8).
All operands SBUF-resident; T=200 recurrence fully unrolled; no per-step DMA.

Per-step structure (per core):
  gates : col-tiled PE MMs — gate n in PE col-group n, psum rows 32n..32n+7.
          k=0 is a one-hot matmul (K=34) against EW = emb@W_x.T + b (bias and
          embedding folded in); k=1..8 contract ctxT/hT feature chunks.
  lstm  : 4 sigmoid/tanh ACTs, 4 DVE STTs, tanh ACT.
  energy: row-form e[1, s-half] per example, col-tiled 4-way over examples
          (ex m -> psum tile m//4, col-group m%4, row 32*(m%4)); exp via
          strided-partition ACT reads compacting to wm[8, 800].
  softmax: masked-mult + row-sum in one DVE STT (accum_out), reciprocal.
  ctx   : wT via 7 PE transposes; row-form ctx accumulated over 7 s-chunks,
          col-tiled 4-way; normalized via scalar.mul with AP scale.
  h/ctx transposed into resident hT_all/ctxT_all (feature-on-partition).
MLP head is hoisted out of the loop: one batched GEMM pass over all T*8
columns at the end (full 128-partition utilization), logits DMA'd per chunk.
"""

import math
import sys
from contextlib import ExitStack

import numpy as np

sys.path.insert(0, "/opt/trn_rl_repo")

import ml_dtypes  # noqa: E402

import concourse.bass as bass  # noqa: E402
import concourse.bacc as bacc  # noqa: E402
import concourse.tile as tile  # noqa: E402
from concourse import mybir  # noqa: E402
from concourse.masks import make_identity  # noqa: E402

BF16 = ml_dtypes.bfloat16

V, E, H, C = 34, 256, 512, 512
B, T, S = 64, 200, 800
NCORES = 8
BL = B // NCORES
SCALE = 1.0 / math.sqrt(128.0)
FDT = mybir.dt.float32
BDT = mybir.dt.bfloat16

S_HALVES = [(0, 512), (512, 288)]  # (offset, width); each fits one psum bank
NSJ = 7  # s chunks of 128 (last has 32 rows)

import os as _os
ABLATE = set(filter(None, _os.environ.get("KABLATE", "").split(",")))


def build_bass(t_steps: int = T) -> bass.Bass:
    nc = bacc.Bacc()

    kts_d = nc.dram_tensor("kts", [BL, 4, 128, S], BDT, kind="ExternalInput")
    v_d = nc.dram_tensor("v", [BL, 6, 128, C], BDT, kind="ExternalInput")
    v6_d = nc.dram_tensor("v6", [128, 2, C], BDT, kind="ExternalInput")
    w_d = nc.dram_tensor("wt", [8, 128, 4 * H], BDT, kind="ExternalInput")
    ew_d = nc.dram_tensor("ewt", [V, 4, 512], BDT, kind="ExternalInput")
    oh_d = nc.dram_tensor("oh", [V, t_steps, BL], BDT, kind="ExternalInput")
    w1_d = nc.dram_tensor("w1t", [8, 128, C], BDT, kind="ExternalInput")
    w2_d = nc.dram_tensor("w2t", [4, 128, V], BDT, kind="ExternalInput")
    b1c_d = nc.dram_tensor("b1c", [128, 4], FDT, kind="ExternalInput")
    b2c_d = nc.dram_tensor("b2c", [V, 1], FDT, kind="ExternalInput")
    mask_d = nc.dram_tensor("mask2", [BL, 2, 512], BDT, kind="ExternalInput")
    out_d = nc.dram_tensor("out", [V, t_steps * BL], BDT, kind="ExternalOutput")

    AF = mybir.ActivationFunctionType
    OP = mybir.AluOpType

    with tile.TileContext(nc) as tc, ExitStack() as es:
        consts = es.enter_context(tc.tile_pool(name="consts", bufs=1))
        state = es.enter_context(tc.tile_pool(name="state", bufs=1))
        work = es.enter_context(tc.tile_pool(name="work", bufs=1))
        work2 = es.enter_context(tc.tile_pool(name="work2", bufs=2))
        peA = es.enter_context(tc.tile_pool(name="peA", bufs=1, space="PSUM"))
        peB = es.enter_context(tc.tile_pool(name="peB", bufs=1, space="PSUM"))
        pt = es.enter_context(tc.tile_pool(name="pt", bufs=3, space="PSUM"))

        # ---- resident inputs ----
        kts_sb = consts.tile([128, BL, 4, S], BDT)
        for ex in range(BL):
            for hk in range(4):
                nc.sync.dma_start(out=kts_sb[:, ex, hk, :], in_=kts_d[ex, hk])
        v_sb = consts.tile([128, BL, 6, C], BDT)
        for ex in range(BL):
            for sj in range(6):
                nc.sync.dma_start(out=v_sb[:, ex, sj, :], in_=v_d[ex, sj])
        v6_sb = consts.tile([128, 2, C], BDT)  # [32*(m%4)+r, m//4, c] = V[m, 768+r, c]
        nc.sync.dma_start(out=v6_sb, in_=v6_d[:, :, :])
        w_sb = consts.tile([128, 8, 4 * H], BDT)
        for k in range(8):
            nc.sync.dma_start(out=w_sb[:, k, :], in_=w_d[k])
        ew_sb = consts.tile([V, 4, 512], BDT)
        nc.sync.dma_start(out=ew_sb, in_=ew_d[:, :, :])
        oh_sb = consts.tile([V, t_steps, BL], BDT)
        nc.sync.dma_start(out=oh_sb, in_=oh_d[:, :, :])
        w1_sb = consts.tile([128, 8, C], BDT)
        for k in range(8):
            nc.sync.dma_start(out=w1_sb[:, k, :], in_=w1_d[k])
        w2_sb = consts.tile([128, 4, V], BDT)
        for k in range(4):
            nc.sync.dma_start(out=w2_sb[:, k, :], in_=w2_d[k])
        b1c_sb = consts.tile([128, 4], FDT)
        nc.sync.dma_start(out=b1c_sb, in_=b1c_d[:, :])
        b2c_sb = consts.tile([V, 1], FDT)
        nc.sync.dma_start(out=b2c_sb, in_=b2c_d[:, :])
        mask_sb = consts.tile([BL, 2, 512], BDT)
        nc.sync.dma_start(out=mask_sb, in_=mask_d[:, :, :])
        id8f = consts.tile([BL, BL], FDT)
        make_identity(nc, id8f)
        id8b = consts.tile([BL, BL], BDT)
        nc.vector.tensor_copy(id8b, id8f)

        # ---- recurrent state: feature-on-partition history ----
        hT_all = state.tile([128, 4, (t_steps + 1) * BL], BDT)
        ctxT_all = state.tile([128, 4, (t_steps + 1) * BL], BDT)
        nc.vector.memset(hT_all[:, :, 0:BL], 0.0)
        nc.vector.memset(ctxT_all[:, :, 0:BL], 0.0)

        P_A = peA.tile([128, 2, 512], FDT, tag="e")
        P_B = peB.tile([128, 2, 512], FDT, tag="e")
        nc.vector.memset(P_A, 0.0)
        nc.vector.memset(P_B, 0.0)

        c_prev = None

        for t in range(t_steps):
            sl_in = slice(t * BL, (t + 1) * BL)
            sl_out = slice((t + 1) * BL, (t + 2) * BL)

            # ---- gates: psum [128, 512]; gate n at rows 32n..32n+7 ----
            # (shares the energy psum banks; strictly ordered by the LSTM chain)
            g_ps = P_A[:, 0, :]
            for k in range(0 if "gates" in ABLATE else 9):
                for n in range(4):
                    if k == 0:
                        lhsT = oh_sb[:, t, :]
                        rhs = ew_sb[:, n, :]
                    elif k <= 4:
                        lhsT = ctxT_all[:, k - 1, sl_in]
                        rhs = w_sb[:, k - 1, n * 512 : (n + 1) * 512]
                    else:
                        lhsT = hT_all[:, k - 5, sl_in]
                        rhs = w_sb[:, k - 1, n * 512 : (n + 1) * 512]
                    nc.tensor.matmul(
                        g_ps[32 * n : 32 * n + BL, :],
                        lhsT,
                        rhs,
                        start=(k == 0),
                        stop=(k == 8),
                        tile_position=(0, 32 * n),
                        skip_group_check=True,
                    )

            # ---- LSTM pointwise ----
            gact = [None] * 4
            for n in (2, 0, 1, 3):  # g first: shortens the c-chain
                ga = work.tile([BL, 512], FDT if n == 1 else BDT, tag=f"ga{n}")
                nc.scalar.activation(
                    ga,
                    g_ps[32 * n : 32 * n + BL, :],
                    AF.Tanh if n == 2 else AF.Sigmoid,
                )
                gact[n] = ga

            def tt(out, a, b, op, accum_out=None):
                nc.vector.scalar_tensor_tensor(
                    out=out, in0=a, scalar=1.0, in1=b, op0=OP.mult, op1=op,
                    accum_out=accum_out,
                )

            t_ig = work.tile([BL, H], BDT, tag="t_ig")
            tt(t_ig, gact[0], gact[2], OP.mult)
            c_new = work2.tile([BL, H], FDT, tag="c")
            if c_prev is None:
                nc.vector.tensor_copy(c_new, t_ig)
            else:
                t_fc = work.tile([BL, H], FDT, tag="t_fc")
                tt(t_fc, gact[1], c_prev, OP.mult)
                tt(c_new, t_ig, t_fc, OP.add)
            c_prev = c_new
            tnc = work.tile([BL, H], BDT, tag="tnc")
            nc.scalar.activation(tnc, c_new, AF.Tanh)
            h_sb = work.tile([BL, H], BDT, tag="h_sb")
            tt(h_sb, gact[3], tnc, OP.mult)

            # ---- h^T into hT_all[slot t+1] ----
            for ck in range(4):
                tp = pt.tile([128, BL], BDT, tag="t")
                nc.tensor.transpose(tp, h_sb[:, ck * 128 : (ck + 1) * 128], id8b)
                nc.vector.tensor_copy(hT_all[:, ck, sl_out], tp)

            # ---- attention energy: ex m -> tile m//4, col-group m%4 ----
            eA = P_A
            eB = P_B
            for hi, (soff, sw) in enumerate([] if "energy" in ABLATE else S_HALVES):
                for hk in range(4):
                    for m in range(BL):
                        g = m % 4
                        e_ps = eA if m < 4 else eB
                        nc.tensor.matmul(
                            e_ps[32 * g : 32 * g + 1, hi, 0:sw],
                            hT_all[:, hk, (t + 1) * BL + m : (t + 1) * BL + m + 1],
                            kts_sb[:, m, hk, soff : soff + sw],
                            start=(hk == 0),
                            stop=(hk == 3),
                            tile_position=(0, 32 * g),
                            skip_group_check=True,
                        )

            # exp over the full 97-row psum block (garbage rows are finite),
            # then DMA partition-gathers compact rows {0,32,64,96} -> 0..3.
            wmA = work.tile([97, 2, 512], BDT, tag="wmA")
            wmB = work.tile([97, 2, 512], BDT, tag="wmB")
            if "exp" not in ABLATE:
                nc.scalar.activation(wmA, eA[0:97, :, :], AF.Exp, scale=SCALE)
                nc.scalar.activation(wmB, eB[0:97, :, :], AF.Exp, scale=SCALE)
            wmc = work.tile([BL, 2, 512], BDT, tag="wmc")
            if "gather" not in ABLATE:
                nc.sync.dma_start(out=wmc[0:4, :, :], in_=wmA[0:97:32, :, :])
                nc.sync.dma_start(out=wmc[4:8, :, :], in_=wmB[0:97:32, :, :])

            # masked weights + row-sum (mask2 zeroes the pad columns too)
            wmask = work.tile([BL, 2, 512], BDT, tag="wmask")
            wsum = work.tile([BL, 1], FDT, tag="wsum")
            tt(wmask, wmc, mask_sb, OP.mult, accum_out=wsum)
            rinv = work.tile([BL, 1], FDT, tag="rinv")
            nc.vector.reciprocal(rinv, wsum)

            # ---- w^T via PE transpose (sj6 lands at rows 32*(m%4)) ----
            wT = work.tile([128, NSJ, BL], BDT, tag="wT")
            for sj in range(0 if "wt" in ABLATE else 6):
                hi, off = (0, sj * 128) if sj < 4 else (1, (sj - 4) * 128)
                tp = pt.tile([128, BL], BDT, tag="t")
                nc.tensor.transpose(tp, wmask[:, hi, off : off + 128], id8b)
                nc.vector.tensor_copy(wT[:, sj, :], tp)
            tp6 = pt.tile([128, BL], BDT, tag="t")
            nc.tensor.transpose(tp6[0:32, :], wmask[:, 1, 256:288], id8b)
            for g in range(4):
                nc.vector.tensor_copy(
                    wT[32 * g : 32 * g + 32, 6, g::4], tp6[0:32, g::4]
                )

            # ---- ctx: row-form, col-tiled; reuses energy psum banks ----
            cA = P_A
            cB = P_B
            for sj in range(0 if "ctx" in ABLATE else NSJ):
                for m in range(BL):
                    g = m % 4
                    c_ps = cA if m < 4 else cB
                    if sj < 6:
                        lhsT = wT[:, sj, m : m + 1]
                        rhs = v_sb[:, m, sj, :]
                        tpos = (0, 32 * g)
                    else:
                        lhsT = wT[32 * g : 32 * g + 32, 6, m : m + 1]
                        rhs = v6_sb[32 * g : 32 * g + 32, m // 4, :]
                        tpos = (32 * g, 32 * g)
                    nc.tensor.matmul(
                        c_ps[32 * g : 32 * g + 1, 0, :],
                        lhsT,
                        rhs,
                        start=(sj == 0),
                        stop=(sj == NSJ - 1),
                        tile_position=tpos,
                        skip_group_check=True,
                    )
            ctxsA = work.tile([97, C], BDT, tag="ctxsA")
            ctxsB = work.tile([97, C], BDT, tag="ctxsB")
            if "ctxcopy" not in ABLATE:
                nc.scalar.copy(ctxsA, cA[0:97, 0, :])
                nc.scalar.copy(ctxsB, cB[0:97, 0, :])
            ctx_u = work.tile([BL, C], BDT, tag="ctx_u")
            if "gather" not in ABLATE:
                nc.sync.dma_start(out=ctx_u[0:4, :], in_=ctxsA[0:97:32, :])
                nc.sync.dma_start(out=ctx_u[4:8, :], in_=ctxsB[0:97:32, :])
            ctx_row = work.tile([BL, C], BDT, tag="ctx_row")
            nc.vector.tensor_scalar_mul(ctx_row, ctx_u, rinv)

            # ---- ctx^T into ctxT_all[slot t+1] ----
            for ck in range(4):
                tp = pt.tile([128, BL], BDT, tag="t")
                nc.tensor.transpose(tp, ctx_row[:, ck * 128 : (ck + 1) * 128], id8b)
                nc.vector.tensor_copy(ctxT_all[:, ck, sl_out], tp)

        # ---- batched MLP head over all t_steps*BL columns ----
        NT = t_steps * BL
        tcs = []
        t0 = 0
        while t0 < NT:
            tcs.append((t0, min(512, NT - t0)))
            t0 += 512
        for ti, (t0, tw) in enumerate(tcs):
            hidT = work.tile([128, 4, 512], BDT, tag="hidT")
            for cj in range(4):
                m_ps = P_A if (ti + cj) % 2 == 0 else P_B
                for k in range(8):
                    src = hT_all if k < 4 else ctxT_all
                    nc.tensor.matmul(
                        m_ps[:, 0, 0:tw],
                        w1_sb[:, k, cj * 128 : (cj + 1) * 128],
                        src[:, k % 4, BL + t0 : BL + t0 + tw],
                        start=(k == 0),
                        stop=(k == 7),
                    )
                nc.scalar.activation(
                    hidT[:, cj, 0:tw],
                    m_ps[:, 0, 0:tw],
                    AF.Tanh,
                    bias=b1c_sb[:, cj : cj + 1],
                )
            l_ps = (P_B if ti % 2 == 0 else P_A)[:, 1, :]
            for ck in range(4):
                nc.tensor.matmul(
                    l_ps[0:V, 0:tw],
                    w2_sb[:, ck, :],
                    hidT[:, ck, 0:tw],
                    start=(ck == 0),
                    stop=(ck == 3),
                )
            o_chunk = work2.tile([V, 512], BDT, tag="o_chunk")
            nc.scalar.activation(
                o_chunk[:, 0:tw], l_ps[0:V, 0:tw], AF.Identity, bias=b2c_sb
            )
            nc.sync.dma_start(out=out_d[:, t0 : t0 + tw], in_=o_chunk[:, 0:tw])

    return nc


def prep_core_inputs(core, tokens, key_enc, value_enc, out_lens, t_steps=T):
    sl = slice(core * BL, (core + 1) * BL)
    ke = key_enc[sl]  # [BL, S, H] f32
    kts = ke.transpose(0, 2, 1).reshape(BL, 4, 128, S).astype(BF16)

    vc = value_enc[sl]  # [BL, S, C]
    v = vc[:, :768].reshape(BL, 6, 128, C).astype(BF16)
    v6 = np.zeros((128, 2, C), np.float32)
    for m in range(BL):
        v6[32 * (m % 4) : 32 * (m % 4) + 32, m // 4] = vc[m, 768:800]

    oh = np.zeros((V, t_steps, BL), np.float32)
    tok = tokens[sl, :t_steps]  # [BL, t]
    for ex in range(BL):
        oh[tok[ex], np.arange(t_steps), ex] = 1.0

    mask = (np.arange(S)[None, :] < out_lens[sl][:, None]).astype(np.float32)
    mask2 = np.zeros((BL, 2, 512), np.float32)
    mask2[:, 0, :] = mask[:, :512]
    mask2[:, 1, :288] = mask[:, 512:]

    return {
        "kts": np.ascontiguousarray(kts),
        "v": np.ascontiguousarray(v),
        "v6": np.ascontiguousarray(v6.astype(BF16)),
        "oh": np.ascontiguousarray(oh.astype(BF16)),
        "mask2": np.ascontiguousarray(mask2.astype(BF16)),
    }


def prep_shared_inputs(emb, W_ih, W_hh, b_ih, b_hh, W1, b1, W2, b2):
    # EW' = emb @ W_x.T + b_ih + b_hh  -> [V, 2048] -> [V, 4, 512]
    ew = emb @ W_ih[:, :E].T + b_ih + b_hh
    wc = np.concatenate([W_ih[:, E:], W_hh], axis=1)  # [2048, 1024]
    wt = wc.T.reshape(8, 128, 4 * H).astype(BF16)
    return {
        "ewt": np.ascontiguousarray(ew.reshape(V, 4, 512).astype(BF16)),
        "wt": np.ascontiguousarray(wt),
        "w1t": np.ascontiguousarray(W1.T.reshape(8, 128, C).astype(BF16)),
        "w2t": np.ascontiguousarray(W2.T.reshape(4, 128, V).astype(BF16)),
        "b1c": np.ascontiguousarray(b1.reshape(4, 128).T.astype(np.float32)),
        "b2c": np.ascontiguousarray(b2[:, None].astype(np.float32)),
    }


_CACHE = {}


def _get_nc(t_steps):
    if t_steps not in _CACHE:
        nc = build_bass(t_steps)
        nc.finalize()
        _CACHE[t_steps] = nc
    return _CACHE[t_steps]


def _build_in_maps(t_steps, inputs):
    args = {k: np.asarray(v) for k, v in inputs.items()}
    tokens = args["tokens"].astype(np.int64)
    shared = prep_shared_inputs(
        args["emb"], args["W_ih"], args["W_hh"], args["b_ih"], args["b_hh"],
        args["W1"], args["b1"], args["W2"], args["b2"],
    )
    in_maps = []
    for core in range(NCORES):
        m = prep_core_inputs(
            core, tokens, args["key_enc"], args["value_enc"], args["out_lens"],
            t_steps=t_steps,
        )
        m.update(shared)
        in_maps.append(m)
    return in_maps


def _unpack_out(outs, t_steps):
    # outs: per-core [V, t_steps*BL] -> [B, t_steps, V]
    full = []
    for o in outs:
        full.append(o.reshape(V, t_steps, BL).transpose(2, 1, 0))
    return np.concatenate(full, axis=0)


def run(t_steps=T, trace=False, **inputs):
    from concourse.bass_utils import run_bass_kernel_spmd

    in_maps = _build_in_maps(t_steps, inputs)
    nc = _get_nc(t_steps)
    res = run_bass_kernel_spmd(nc, in_maps, list(range(NCORES)), trace=trace)
    outs = [np.asarray(r["out"], np.float32) for r in res.results]
    return _unpack_out(outs, t_steps), res


def kernel(**inputs) -> np.ndarray:
    full, _ = run(t_steps=T, trace=False, **inputs)
    return full


def warm_timing(t_steps=T, n_iters=12, **inputs):
    """Time warm NEFF executions (device-resident inputs) as an HW-time proxy."""
    import time

    import jax
    from jax.sharding import Mesh, PartitionSpec
    from jax.experimental.shard_map import shard_map

    from concourse import bass2jax
    from concourse import mybir as _mybir
    from concourse.bass2jax import _bass_exec_p, install_neuronx_cc_hook

    install_neuronx_cc_hook()
    in_maps = _build_in_maps(t_steps, inputs)
    nc = _get_nc(t_steps)

    partition_name = nc.partition_id_tensor.name if nc.partition_id_tensor else None
    in_names, out_names, out_avals, zero_outs = [], [], [], []
    for alloc in nc.m.functions[0].allocations:
        if not isinstance(alloc, _mybir.MemoryLocationSet):
            continue
        name = alloc.memorylocations[0].name
        if alloc.kind == "ExternalInput":
            if name != partition_name:
                in_names.append(name)
        elif alloc.kind == "ExternalOutput":
            out_names.append(name)
            shape = tuple(alloc.tensor_shape)
            dtype = _mybir.dt.np(alloc.dtype)
            out_avals.append(jax.core.ShapedArray(shape, dtype))
            zero_outs.append(np.zeros(shape, dtype))
    n_params = len(in_names)
    n_outs = len(out_avals)
    in_names.extend(out_names)
    if partition_name:
        in_names.append(partition_name)

    def _body(*a):
        operands = list(a)
        if partition_name:
            operands.append(bass2jax.partition_id_tensor())
        return tuple(
            _bass_exec_p.bind(
                *operands,
                out_avals=tuple(out_avals),
                in_names=tuple(in_names),
                out_names=tuple(out_names),
                lowering_input_output_aliases=(),
                sim_require_finite=True,
                sim_require_nnan=True,
                nc=nc,
            )
        )

    devices = jax.devices()[:NCORES]
    mesh = Mesh(np.asarray(devices), ("core",))
    sharded = jax.jit(
        shard_map(
            _body,
            mesh=mesh,
            in_specs=(PartitionSpec("core"),) * (n_params + n_outs),
            out_specs=(PartitionSpec("core"),) * len(out_names),
            check_rep=False,
        ),
        keep_unused=True,
    )
    per_core = [[np.asarray(m[nm]) for nm in in_names[:n_params]] for m in in_maps]
    concat_in = [
        jax.device_put(np.concatenate([per_core[c][i] for c in range(NCORES)], axis=0))
        for i in range(n_params)
    ]
    concat_zeros = [
        jax.device_put(np.zeros((NCORES * z.shape[0], *z.shape[1:]), z.dtype))
        for z in zero_outs
    ]
    outs = sharded(*concat_in, *concat_zeros)
    jax.block_until_ready(outs)
    best = None
    for _ in range(n_iters):
        t0 = time.time()
        outs = sharded(*concat_in, *concat_zeros)
        jax.block_until_ready(outs)
        dt = time.time() - t0
        best = dt if best is None else min(best, dt)

    oarr = np.asarray(outs[out_names.index("out")]).reshape(NCORES, V, t_steps * BL)
    full = _unpack_out(list(oarr), t_steps)
    return best, full


# revision 6
# speedup vs baseline: 1.3026x; 1.2400x over previous
"""Attention-LSTM decoder (nn_Decoder) Trainium2 Bass kernel, v2.

Sharding: data-parallel over batch B=64 -> 8 cores x 8 examples (BL=8).
All operands SBUF-resident; T=200 recurrence fully unrolled; no per-step DMA.

Per-step structure (per core):
  gates : col-tiled PE MMs — gate n in PE col-group n, psum rows 32n..32n+7.
          k=0 is a one-hot matmul (K=34) against EW = emb@W_x.T + b (bias and
          embedding folded in); k=1..8 contract ctxT/hT feature chunks.
  lstm  : 4 sigmoid/tanh ACTs, 4 DVE STTs, tanh ACT.
  energy: row-form e[1, s-half] per example, col-tiled 4-way over examples
          (ex m -> psum tile m//4, col-group m%4, row 32*(m%4)); exp via
          strided-partition ACT reads compacting to wm[8, 800].
  softmax: masked-mult + row-sum in one DVE STT (accum_out), reciprocal.
  ctx   : wT via 7 PE transposes; row-form ctx accumulated over 7 s-chunks,
          col-tiled 4-way; normalized via scalar.mul with AP scale.
  h/ctx transposed into resident hT_all/ctxT_all (feature-on-partition).
MLP head is hoisted out of the loop: one batched GEMM pass over all T*8
columns at the end (full 128-partition utilization), logits DMA'd per chunk.
"""

import math
import sys
from contextlib import ExitStack

import numpy as np

sys.path.insert(0, "/opt/trn_rl_repo")

import ml_dtypes  # noqa: E402

import concourse.bass as bass  # noqa: E402
import concourse.bacc as bacc  # noqa: E402
import concourse.tile as tile  # noqa: E402
from concourse import mybir  # noqa: E402
from concourse.masks import make_identity  # noqa: E402

BF16 = ml_dtypes.bfloat16

V, E, H, C = 34, 256, 512, 512
B, T, S = 64, 200, 800
NCORES = 8
BL = B // NCORES
SCALE = 1.0 / math.sqrt(128.0)
FDT = mybir.dt.float32
BDT = mybir.dt.bfloat16

S_HALVES = [(0, 512), (512, 288)]  # (offset, width); each fits one psum bank
NSJ = 7  # s chunks of 128 (last has 32 rows)

import os as _os
ABLATE = set(filter(None, _os.environ.get("KABLATE", "").split(",")))


def build_bass(t_steps: int = T) -> bass.Bass:
    nc = bacc.Bacc()

    kts_d = nc.dram_tensor("kts", [BL, 4, 128, S], BDT, kind="ExternalInput")
    v_d = nc.dram_tensor("v", [BL, 6, 128, C], BDT, kind="ExternalInput")
    v6_d = nc.dram_tensor("v6", [128, 2, C], BDT, kind="ExternalInput")
    w_d = nc.dram_tensor("wt", [8, 128, 4 * H], BDT, kind="ExternalInput")
    ew_d = nc.dram_tensor("ewt", [V, 4, 512], BDT, kind="ExternalInput")
    oh_d = nc.dram_tensor("oh", [V, t_steps, BL], BDT, kind="ExternalInput")
    w1_d = nc.dram_tensor("w1t", [8, 128, C], BDT, kind="ExternalInput")
    w2_d = nc.dram_tensor("w2t", [4, 128, V], BDT, kind="ExternalInput")
    b1c_d = nc.dram_tensor("b1c", [128, 4], FDT, kind="ExternalInput")
    b2c_d = nc.dram_tensor("b2c", [V, 1], FDT, kind="ExternalInput")
    mask_d = nc.dram_tensor("mask2", [BL, 2, 512], BDT, kind="ExternalInput")
    out_d = nc.dram_tensor("out", [V, t_steps * BL], BDT, kind="ExternalOutput")

    AF = mybir.ActivationFunctionType
    OP = mybir.AluOpType

    with tile.TileContext(nc) as tc, ExitStack() as es:
        consts = es.enter_context(tc.tile_pool(name="consts", bufs=1))
        state = es.enter_context(tc.tile_pool(name="state", bufs=1))
        work = es.enter_context(tc.tile_pool(name="work", bufs=1))
        work2 = es.enter_context(tc.tile_pool(name="work2", bufs=2))
        peA = es.enter_context(tc.tile_pool(name="peA", bufs=1, space="PSUM"))
        peB = es.enter_context(tc.tile_pool(name="peB", bufs=1, space="PSUM"))
        ps_sig = es.enter_context(tc.tile_pool(name="ps_sig", bufs=1, space="PSUM"))
        pt = es.enter_context(tc.tile_pool(name="pt", bufs=3, space="PSUM"))

        # ---- resident inputs ----
        kts_sb = consts.tile([128, BL, 4, S], BDT)
        for ex in range(BL):
            for hk in range(4):
                nc.sync.dma_start(out=kts_sb[:, ex, hk, :], in_=kts_d[ex, hk])
        v_sb = consts.tile([128, BL, 6, C], BDT)
        for ex in range(BL):
            for sj in range(6):
                nc.sync.dma_start(out=v_sb[:, ex, sj, :], in_=v_d[ex, sj])
        v6_sb = consts.tile([128, 2, C], BDT)  # [32*(m%4)+r, m//4, c] = V[m, 768+r, c]
        nc.sync.dma_start(out=v6_sb, in_=v6_d[:, :, :])
        w_sb = consts.tile([128, 8, 4 * H], BDT)
        for k in range(8):
            nc.sync.dma_start(out=w_sb[:, k, :], in_=w_d[k])
        ew_sb = consts.tile([V, 4, 512], BDT)
        nc.sync.dma_start(out=ew_sb, in_=ew_d[:, :, :])
        oh_sb = consts.tile([V, t_steps, BL], BDT)
        nc.sync.dma_start(out=oh_sb, in_=oh_d[:, :, :])
        w1_sb = consts.tile([128, 8, C], BDT)
        for k in range(8):
            nc.sync.dma_start(out=w1_sb[:, k, :], in_=w1_d[k])
        w2_sb = consts.tile([128, 4, V], BDT)
        for k in range(4):
            nc.sync.dma_start(out=w2_sb[:, k, :], in_=w2_d[k])
        b1c_sb = consts.tile([128, 4], FDT)
        nc.sync.dma_start(out=b1c_sb, in_=b1c_d[:, :])
        b2c_sb = consts.tile([V, 1], FDT)
        nc.sync.dma_start(out=b2c_sb, in_=b2c_d[:, :])
        mask_sb = consts.tile([BL, 2, 512], BDT)
        nc.sync.dma_start(out=mask_sb, in_=mask_d[:, :, :])
        id8f = consts.tile([BL, BL], FDT)
        make_identity(nc, id8f)
        id8b = consts.tile([BL, BL], BDT)
        nc.vector.tensor_copy(id8b, id8f)

        # ---- recurrent state: feature-on-partition history ----
        hT_all = state.tile([128, 4, (t_steps + 1) * BL], BDT)
        ctxT_all = state.tile([128, 4, (t_steps + 1) * BL], BDT)
        nc.vector.memset(hT_all[:, :, 0:BL], 0.0)
        nc.vector.memset(ctxT_all[:, :, 0:BL], 0.0)

        P_A = peA.tile([128, 2, 512], FDT, tag="e")
        P_B = peB.tile([128, 2, 512], FDT, tag="e")
        P_S = ps_sig.tile([72, 512], FDT, tag="sig")
        nc.vector.memset(P_A, 0.0)
        nc.vector.memset(P_B, 0.0)

        c_prev = None

        def gates_mms(t, ks):
            g_ps = P_A[:, 0, :]
            for k in ks:
                for n in range(4):
                    if k == 0:
                        lhsT = oh_sb[:, t, :]
                        rhs = ew_sb[:, n, :]
                    elif k <= 4:
                        lhsT = ctxT_all[:, k - 1, t * BL : (t + 1) * BL]
                        rhs = w_sb[:, k - 1, n * 512 : (n + 1) * 512]
                    else:
                        lhsT = hT_all[:, k - 5, t * BL : (t + 1) * BL]
                        rhs = w_sb[:, k - 1, n * 512 : (n + 1) * 512]
                    nc.tensor.matmul(
                        g_ps[32 * n : 32 * n + BL, :],
                        lhsT,
                        rhs,
                        start=(k == 0),
                        stop=(k == 4),
                        tile_position=(0, 32 * n),
                        skip_group_check=True,
                    )

        if "gates" not in ABLATE:
            gates_mms(0, (0, 5, 6, 7, 8))

        for t in range(t_steps):
            sl_in = slice(t * BL, (t + 1) * BL)
            sl_out = slice((t + 1) * BL, (t + 2) * BL)

            # ---- gates (finish): ctx-chunk MMs; the onehot+h chunks were
            # issued at the end of the previous step (overlap its ctxT tail)
            g_ps = P_A[:, 0, :]
            if "gates" not in ABLATE:
                gates_mms(t, (1, 2, 3, 4))
                if t == 0:
                    pass

            # ---- LSTM pointwise ----
            # gate order in W is (i, f, o, g): one sigmoid ACT covers i,f,o
            ga_g = work.tile([BL, 512], BDT, tag="ga_g")
            nc.scalar.activation(ga_g, g_ps[96 : 96 + BL, :], AF.Tanh)
            nc.scalar.activation(P_S, g_ps[0:72, :], AF.Sigmoid)
            s_i = P_S[0:BL, :]
            s_f = P_S[32 : 32 + BL, :]
            s_o = P_S[64 : 64 + BL, :]

            def tt(out, a, b, op, accum_out=None):
                nc.vector.scalar_tensor_tensor(
                    out=out, in0=a, scalar=1.0, in1=b, op0=OP.mult, op1=op,
                    accum_out=accum_out,
                )

            t_ig = work.tile([BL, H], BDT, tag="t_ig")
            tt(t_ig, s_i, ga_g, OP.mult)
            c_new = work2.tile([BL, H], FDT, tag="c")
            if c_prev is None:
                nc.vector.tensor_copy(c_new, t_ig)
            else:
                t_fc = work.tile([BL, H], FDT, tag="t_fc")
                tt(t_fc, s_f, c_prev, OP.mult)
                tt(c_new, t_ig, t_fc, OP.add)
            c_prev = c_new
            tnc = work.tile([BL, H], BDT, tag="tnc")
            nc.scalar.activation(tnc, c_new, AF.Tanh)
            h_sb = work.tile([BL, H], BDT, tag="h_sb")
            tt(h_sb, s_o, tnc, OP.mult)

            # ---- h^T into hT_all[slot t+1] ----
            for ck in range(4):
                tp = pt.tile([128, BL], BDT, tag="t")
                nc.tensor.transpose(tp, h_sb[:, ck * 128 : (ck + 1) * 128], id8b)
                nc.vector.tensor_copy(hT_all[:, ck, sl_out], tp)

            # ---- attention energy: ex m -> tile m//4, col-group m%4 ----
            eA = P_A
            eB = P_B
            wmA = work.tile([97, 2, 512], BDT, tag="wmA")
            wmB = work.tile([97, 2, 512], BDT, tag="wmB")

            def energy_block(hi, soff, sw, ms, e_ps):
                for hk in range(4):
                    for m in ms:
                        g = m % 4
                        nc.tensor.matmul(
                            e_ps[32 * g : 32 * g + 1, hi, 0:sw],
                            hT_all[:, hk, (t + 1) * BL + m : (t + 1) * BL + m + 1],
                            kts_sb[:, m, hk, soff : soff + sw],
                            start=(hk == 0),
                            stop=(hk == 3),
                            tile_position=(0, 32 * g),
                            skip_group_check=True,
                        )

            if "energy" not in ABLATE:
                # block order lets each half's exp ACT overlap later MM blocks
                energy_block(0, 0, 512, range(0, 4), eA)
                energy_block(0, 0, 512, range(4, 8), eB)
                nc.scalar.activation(
                    wmA[:, 0, :], eA[0:97, 0, :], AF.Exp, scale=SCALE
                )
                energy_block(1, 512, 288, range(0, 4), eA)
                nc.scalar.activation(
                    wmB[:, 0, :], eB[0:97, 0, :], AF.Exp, scale=SCALE
                )
                energy_block(1, 512, 288, range(4, 8), eB)
                nc.scalar.activation(
                    wmA[:, 1, :], eA[0:97, 1, :], AF.Exp, scale=SCALE
                )
                nc.scalar.activation(
                    wmB[:, 1, :], eB[0:97, 1, :], AF.Exp, scale=SCALE
                )
            wmc = work.tile([BL, 2, 512], BDT, tag="wmc")
            if "gather" not in ABLATE:
                nc.sync.dma_start(out=wmc[0:4, :, :], in_=wmA[0:97:32, :, :])
                nc.sync.dma_start(out=wmc[4:8, :, :], in_=wmB[0:97:32, :, :])

            # masked weights + row-sum (mask2 zeroes the pad columns too)
            wmask = work.tile([BL, 2, 512], BDT, tag="wmask")
            wsum = work.tile([BL, 1], FDT, tag="wsum")
            tt(wmask, wmc, mask_sb, OP.mult, accum_out=wsum)
            rinv = work.tile([BL, 1], FDT, tag="rinv")
            nc.vector.reciprocal(rinv, wsum)

            # ---- w^T transposes interleaved with ctx MM rounds ----
            wT = work.tile([128, NSJ, BL], BDT, tag="wT")
            cA = P_A
            cB = P_B

            def ctx_round(sj):
                for m in range(BL):
                    g = m % 4
                    c_ps = cA if m < 4 else cB
                    if sj < 6:
                        lhsT = wT[:, sj, m : m + 1]
                        rhs = v_sb[:, m, sj, :]
                        tpos = (0, 32 * g)
                    else:
                        lhsT = wT[32 * g : 32 * g + 32, 6, m : m + 1]
                        rhs = v6_sb[32 * g : 32 * g + 32, m // 4, :]
                        tpos = (32 * g, 32 * g)
                    nc.tensor.matmul(
                        c_ps[32 * g : 32 * g + 1, 0, :],
                        lhsT,
                        rhs,
                        start=(sj == 0),
                        stop=(sj == NSJ - 1),
                        tile_position=tpos,
                        skip_group_check=True,
                    )

            if "ctx" not in ABLATE:
                for sj in range(6):
                    hi, off = (0, sj * 128) if sj < 4 else (1, (sj - 4) * 128)
                    tp = pt.tile([128, BL], BDT, tag="t")
                    nc.tensor.transpose(tp, wmask[:, hi, off : off + 128], id8b)
                    nc.vector.tensor_copy(wT[:, sj, :], tp)
                tp6 = pt.tile([128, BL], BDT, tag="t")
                nc.tensor.transpose(tp6[0:32, :], wmask[:, 1, 256:288], id8b)
                for g in range(4):
                    nc.vector.tensor_copy(
                        wT[32 * g : 32 * g + 32, 6, g::4], tp6[0:32, g::4]
                    )
                for sj in range(NSJ):
                    ctx_round(sj)
            ctxsA = work.tile([97, C], BDT, tag="ctxsA")
            ctxsB = work.tile([97, C], BDT, tag="ctxsB")
            if "ctxcopy" not in ABLATE:
                nc.scalar.copy(ctxsA, cA[0:97, 0, :])
                nc.scalar.copy(ctxsB, cB[0:97, 0, :])
            ctx_u = work.tile([BL, C], BDT, tag="ctx_u")
            if "gather" not in ABLATE:
                nc.sync.dma_start(out=ctx_u[0:4, :], in_=ctxsA[0:97:32, :])
                nc.sync.dma_start(out=ctx_u[4:8, :], in_=ctxsB[0:97:32, :])
            ctx_row = work.tile([BL, C], BDT, tag="ctx_row")
            nc.vector.tensor_scalar_mul(ctx_row, ctx_u, rinv)

            # start next step's gates (onehot + h chunks) before the ctxT tail
            if t + 1 < t_steps and "gates" not in ABLATE:
                gates_mms(t + 1, (0, 5, 6, 7, 8))

            # ---- ctx^T into ctxT_all[slot t+1] ----
            for ck in range(4):
                tp = pt.tile([128, BL], BDT, tag="t")
                nc.tensor.transpose(tp, ctx_row[:, ck * 128 : (ck + 1) * 128], id8b)
                nc.vector.tensor_copy(ctxT_all[:, ck, sl_out], tp)

        # ---- batched MLP head over all t_steps*BL columns ----
        NT = t_steps * BL
        tcs = []
        t0 = 0
        while t0 < NT:
            tcs.append((t0, min(512, NT - t0)))
            t0 += 512
        for ti, (t0, tw) in enumerate(tcs):
            hidT = work.tile([128, 4, 512], BDT, tag="hidT")
            for cj in range(4):
                m_ps = P_A if (ti + cj) % 2 == 0 else P_B
                for k in range(8):
                    src = hT_all if k < 4 else ctxT_all
                    nc.tensor.matmul(
                        m_ps[:, 0, 0:tw],
                        w1_sb[:, k, cj * 128 : (cj + 1) * 128],
                        src[:, k % 4, BL + t0 : BL + t0 + tw],
                        start=(k == 0),
                        stop=(k == 7),
                    )
                nc.scalar.activation(
                    hidT[:, cj, 0:tw],
                    m_ps[:, 0, 0:tw],
                    AF.Tanh,
                    bias=b1c_sb[:, cj : cj + 1],
                )
            l_ps = (P_B if ti % 2 == 0 else P_A)[:, 1, :]
            for ck in range(4):
                nc.tensor.matmul(
                    l_ps[0:V, 0:tw],
                    w2_sb[:, ck, :],
                    hidT[:, ck, 0:tw],
                    start=(ck == 0),
                    stop=(ck == 3),
                )
            o_chunk = work2.tile([V, 512], BDT, tag="o_chunk")
            nc.scalar.activation(
                o_chunk[:, 0:tw], l_ps[0:V, 0:tw], AF.Identity, bias=b2c_sb
            )
            nc.sync.dma_start(out=out_d[:, t0 : t0 + tw], in_=o_chunk[:, 0:tw])

    return nc


def prep_core_inputs(core, tokens, key_enc, value_enc, out_lens, t_steps=T):
    sl = slice(core * BL, (core + 1) * BL)
    ke = key_enc[sl]  # [BL, S, H] f32
    kts = ke.transpose(0, 2, 1).reshape(BL, 4, 128, S).astype(BF16)

    vc = value_enc[sl]  # [BL, S, C]
    v = vc[:, :768].reshape(BL, 6, 128, C).astype(BF16)
    v6 = np.zeros((128, 2, C), np.float32)
    for m in range(BL):
        v6[32 * (m % 4) : 32 * (m % 4) + 32, m // 4] = vc[m, 768:800]

    oh = np.zeros((V, t_steps, BL), np.float32)
    tok = tokens[sl, :t_steps]  # [BL, t]
    for ex in range(BL):
        oh[tok[ex], np.arange(t_steps), ex] = 1.0

    mask = (np.arange(S)[None, :] < out_lens[sl][:, None]).astype(np.float32)
    mask2 = np.zeros((BL, 2, 512), np.float32)
    mask2[:, 0, :] = mask[:, :512]
    mask2[:, 1, :288] = mask[:, 512:]

    return {
        "kts": np.ascontiguousarray(kts),
        "v": np.ascontiguousarray(v),
        "v6": np.ascontiguousarray(v6.astype(BF16)),
        "oh": np.ascontiguousarray(oh.astype(BF16)),
        "mask2": np.ascontiguousarray(mask2.astype(BF16)),
    }


def prep_shared_inputs(emb, W_ih, W_hh, b_ih, b_hh, W1, b1, W2, b2):
    # EW' = emb @ W_x.T + b_ih + b_hh  -> [V, 2048] -> [V, 4, 512]
    # gate blocks reordered (i, f, o, g) so one sigmoid ACT covers rows 0..71
    perm = np.r_[0:1024, 1536:2048, 1024:1536]
    ew = (emb @ W_ih[:, :E].T + b_ih + b_hh)[:, perm]
    wc = np.concatenate([W_ih[:, E:], W_hh], axis=1)[perm]  # [2048, 1024]
    wt = wc.T.reshape(8, 128, 4 * H).astype(BF16)
    return {
        "ewt": np.ascontiguousarray(ew.reshape(V, 4, 512).astype(BF16)),
        "wt": np.ascontiguousarray(wt),
        "w1t": np.ascontiguousarray(W1.T.reshape(8, 128, C).astype(BF16)),
        "w2t": np.ascontiguousarray(W2.T.reshape(4, 128, V).astype(BF16)),
        "b1c": np.ascontiguousarray(b1.reshape(4, 128).T.astype(np.float32)),
        "b2c": np.ascontiguousarray(b2[:, None].astype(np.float32)),
    }


_CACHE = {}


def _get_nc(t_steps):
    if t_steps not in _CACHE:
        nc = build_bass(t_steps)
        nc.finalize()
        _CACHE[t_steps] = nc
    return _CACHE[t_steps]


def _build_in_maps(t_steps, inputs):
    args = {k: np.asarray(v) for k, v in inputs.items()}
    tokens = args["tokens"].astype(np.int64)
    shared = prep_shared_inputs(
        args["emb"], args["W_ih"], args["W_hh"], args["b_ih"], args["b_hh"],
        args["W1"], args["b1"], args["W2"], args["b2"],
    )
    in_maps = []
    for core in range(NCORES):
        m = prep_core_inputs(
            core, tokens, args["key_enc"], args["value_enc"], args["out_lens"],
            t_steps=t_steps,
        )
        m.update(shared)
        in_maps.append(m)
    return in_maps


def _unpack_out(outs, t_steps):
    # outs: per-core [V, t_steps*BL] -> [B, t_steps, V]
    full = []
    for o in outs:
        full.append(o.reshape(V, t_steps, BL).transpose(2, 1, 0))
    return np.concatenate(full, axis=0)


def run(t_steps=T, trace=False, **inputs):
    from concourse.bass_utils import run_bass_kernel_spmd

    in_maps = _build_in_maps(t_steps, inputs)
    nc = _get_nc(t_steps)
    res = run_bass_kernel_spmd(nc, in_maps, list(range(NCORES)), trace=trace)
    outs = [np.asarray(r["out"], np.float32) for r in res.results]
    return _unpack_out(outs, t_steps), res


def kernel(**inputs) -> np.ndarray:
    full, _ = run(t_steps=T, trace=False, **inputs)
    return full


def warm_timing(t_steps=T, n_iters=12, **inputs):
    """Time warm NEFF executions (device-resident inputs) as an HW-time proxy."""
    import time

    import jax
    from jax.sharding import Mesh, PartitionSpec
    from jax.experimental.shard_map import shard_map

    from concourse import bass2jax
    from concourse import mybir as _mybir
    from concourse.bass2jax import _bass_exec_p, install_neuronx_cc_hook

    install_neuronx_cc_hook()
    in_maps = _build_in_maps(t_steps, inputs)
    nc = _get_nc(t_steps)

    partition_name = nc.partition_id_tensor.name if nc.partition_id_tensor else None
    in_names, out_names, out_avals, zero_outs = [], [], [], []
    for alloc in nc.m.functions[0].allocations:
        if not isinstance(alloc, _mybir.MemoryLocationSet):
            continue
        name = alloc.memorylocations[0].name
        if alloc.kind == "ExternalInput":
            if name != partition_name:
                in_names.append(name)
        elif alloc.kind == "ExternalOutput":
            out_names.append(name)
            shape = tuple(alloc.tensor_shape)
            dtype = _mybir.dt.np(alloc.dtype)
            out_avals.append(jax.core.ShapedArray(shape, dtype))
            zero_outs.append(np.zeros(shape, dtype))
    n_params = len(in_names)
    n_outs = len(out_avals)
    in_names.extend(out_names)
    if partition_name:
        in_names.append(partition_name)

    def _body(*a):
        operands = list(a)
        if partition_name:
            operands.append(bass2jax.partition_id_tensor())
        return tuple(
            _bass_exec_p.bind(
                *operands,
                out_avals=tuple(out_avals),
                in_names=tuple(in_names),
                out_names=tuple(out_names),
                lowering_input_output_aliases=(),
                sim_require_finite=True,
                sim_require_nnan=True,
                nc=nc,
            )
        )

    devices = jax.devices()[:NCORES]
    mesh = Mesh(np.asarray(devices), ("core",))
    sharded = jax.jit(
        shard_map(
            _body,
            mesh=mesh,
            in_specs=(PartitionSpec("core"),) * (n_params + n_outs),
            out_specs=(PartitionSpec("core"),) * len(out_names),
            check_rep=False,
        ),
        keep_unused=True,
    )
    per_core = [[np.asarray(m[nm]) for nm in in_names[:n_params]] for m in in_maps]
    concat_in = [
        jax.device_put(np.concatenate([per_core[c][i] for c in range(NCORES)], axis=0))
        for i in range(n_params)
    ]
    concat_zeros = [
        jax.device_put(np.zeros((NCORES * z.shape[0], *z.shape[1:]), z.dtype))
        for z in zero_outs
    ]
    outs = sharded(*concat_in, *concat_zeros)
    jax.block_until_ready(outs)
    best = None
    for _ in range(n_iters):
        t0 = time.time()
        outs = sharded(*concat_in, *concat_zeros)
        jax.block_until_ready(outs)
        dt = time.time() - t0
        best = dt if best is None else min(best, dt)

    oarr = np.asarray(outs[out_names.index("out")]).reshape(NCORES, V, t_steps * BL)
    full = _unpack_out(list(oarr), t_steps)
    return best, full
